# revision 7
# baseline (speedup 1.0000x reference)
"""CKSAAP embedding-average histogram kernel for Trainium2 (8 NeuronCores).

Problem: seq [B=32, L=4096] int codes in [0,20); emb [B, L, D=64] f32; k=7.
out[b, t, a, c, :] = (1/(L-t-1)) * sum_j 0.5*(emb[b,j]+emb[b,j+t+1])
                     over j with seq[b,j]==a and seq[b,j+t+1]==c.

Sharding: data-parallel over batch, 4 batches per core.

Device algorithm (per core), pure one-hot matmul formulation:
  For each (local batch q, gap t): V = emb[j] + emb[j+s]  (s=t+1, f32, DVE),
  split V into bf16 hi+lo halves (exact to ~2^-17), build the pair-code
  one-hot P [positions, 401] on DVE (bf16 is_equal against a remapped iota
  row; code 400 is a junk bin absorbing the invalid tail), then TensorE
  accumulates psum[128, 401] = [V_hi; V_lo]^T @ P over 32 position-chunks
  of 128 (chunk c = positions {32p + c}, so emb loads stay natural/contiguous
  with an 8-column overlap for the shifts).  Eviction sums hi+lo rows and
  scales by 0.5/(L-s) on ScalarE.  Output is written transposed [64, 400];
  the host restores [400, 64] ordering when unsharding.

Codes are remapped by g(n) = n (n<=255) else 256 + 4*(n-256) so every code
is exactly representable in bf16 (no is_equal collisions).
"""

import numpy as np
import ml_dtypes

NUM_AA = 20
L = 4096
D = 64
T = 8           # gaps t = 0..7 (k+1)
B = 32
NCORES = 8
BPC = B // NCORES   # batches per core
NBINS = NUM_AA * NUM_AA  # 400
NCH = 32        # contraction chunks of 128 positions
OVC = 40        # overlap columns per partition (32 + max shift 8)
NCOLS = NBINS + 1  # 401, junk bin last
IOTA_W = 416    # padded iota row width (32-aligned keeps DVE fast mode)

_CACHE = {}


def _g(n):
    """Injective map of codes 0..400 onto exactly-bf16-representable floats."""
    n = np.asarray(n, dtype=np.int64)
    return np.where(n <= 255, n, 256 + 4 * (n - 256)).astype(np.float32)


def _build(nbt=BPC * T):
    """Build the per-core Bass program. nbt = number of (batch, t) combos."""
    key = ("nc", nbt)
    if key in _CACHE:
        return _CACHE[key]
    import concourse.bass as bass
    import concourse.bacc as bacc
    import concourse.mybir as mybir
    from concourse import tile

    fp32 = mybir.dt.float32
    bf16 = mybir.dt.bfloat16
    AOP = mybir.AluOpType
    ACT = mybir.ActivationFunctionType

    nc = bacc.Bacc()
    emb_ov = nc.dram_tensor("emb_ov", [BPC, 128, OVC * D], fp32, kind="ExternalInput")
    keys = nc.dram_tensor("keys", [nbt, 128, NCH], fp32, kind="ExternalInput")
    iota = nc.dram_tensor("iota", [128, IOTA_W], bf16, kind="ExternalInput")
    outp = nc.dram_tensor("outp", [nbt, D, NBINS], fp32, kind="ExternalOutput")

    with tile.TileContext(nc) as tc:
        with (
            tc.tile_pool(name="const", bufs=1) as cpool,
            tc.tile_pool(name="work", bufs=2) as wpool,
            tc.tile_pool(name="vwork", bufs=3) as vpool,
            tc.tile_pool(name="outs", bufs=3) as opool,
            tc.tile_pool(name="psum", bufs=6, space="PSUM") as ppool,
        ):
            emb_sb = cpool.tile([128, BPC * OVC * D], fp32, tag="emb")
            keys_sb = cpool.tile([128, nbt, NCH], fp32, tag="keys")
            iota_sb = cpool.tile([128, IOTA_W], bf16, tag="iota")

            for q in range(BPC):
                nc.sync.dma_start(
                    emb_sb[:, q * OVC * D:(q + 1) * OVC * D], emb_ov[q]
                )
            nc.sync.dma_start(keys_sb[:], keys[:].rearrange("bt p c -> p bt c"))
            nc.sync.dma_start(iota_sb[:], iota[:])

            for bt in range(nbt):
                q, t = divmod(bt, T)
                s = t + 1
                n_t = L - s
                scale = float(0.5 / n_t)
                base = q * OVC * D

                vf = vpool.tile([128, NCH * D], fp32, tag="vf")
                vhl = vpool.tile([128, NCH, 128], bf16, tag="vhl")
                pall = wpool.tile([128, NCH, IOTA_W], bf16, tag="pall")

                # V = e_j + e_{j+s}; position j = 32p + c lives at [p, c*64:...]
                nc.vector.tensor_tensor(
                    vf[:],
                    emb_sb[:, base:base + NCH * D],
                    emb_sb[:, base + s * D:base + s * D + NCH * D],
                    AOP.add,
                )
                vf3 = vf[:].rearrange("p (c d) -> p c d", c=NCH)
                # hi = bf16(V) on ScalarE
                nc.scalar.activation(vhl[:, :, 0:D], vf3, ACT.Copy)
                # lo = bf16(V - hi) on GpSimd (keeps DVE free for P-gen)
                nc.gpsimd.tensor_tensor(
                    vhl[:, :, D:2 * D], vf3, vhl[:, :, 0:D], AOP.subtract
                )
                # P[p, c, n] = (iota[n] == keys[p, bt*NCH + c]) in bf16.
                # One tensor_scalar per chunk: single-src + bf16 + unit stride
                # hits the DVE 4x perf mode (a broadcast tensor_tensor runs 1x).
                for c in range(NCH):
                    nc.vector.tensor_scalar(
                        pall[:, c, :],
                        iota_sb[:],
                        keys_sb[:, bt, c:c + 1],
                        None,
                        AOP.is_equal,
                    )

                ps = ppool.tile([128, NCOLS], fp32, tag="ps")
                for c in range(NCH):
                    nc.tensor.matmul(
                        ps[:],
                        vhl[:, c, :],
                        pall[:, c, 0:NCOLS],
                        start=(c == 0),
                        stop=(c == NCH - 1),
                    )

                # DVE cannot mix partition bases across operands; ScalarE can
                # read an offset partition base, so evict hi/lo separately
                # (scale folded into each) and add on DVE.
                his = opool.tile([D, NBINS], fp32, tag="his")
                los = opool.tile([D, NBINS], fp32, tag="los")
                oscaled = opool.tile([D, NBINS], fp32, tag="oscaled")
                nc.scalar.activation(his[:], ps[0:D, 0:NBINS], ACT.Copy, scale=scale)
                nc.scalar.activation(los[:], ps[D:128, 0:NBINS], ACT.Copy, scale=scale)
                nc.vector.tensor_tensor(oscaled[:], his[:], los[:], AOP.add)
                nc.sync.dma_start(outp[bt], oscaled[:])

    nc.finalize()
    _CACHE[key] = nc
    return nc


def _host_prep(seq_np, emb_np, core):
    """Build the per-core input arrays for core index `core`."""
    q0 = core * BPC
    emb_c = emb_np[q0:q0 + BPC]  # [BPC, L, D] f32
    # overlap layout: emb_ov[q, p, :] = emb[q, 32p : 32p+40, :] (zero-padded)
    embp = np.zeros((BPC, L + OVC - 32, D), np.float32)
    embp[:, :L] = emb_c
    idx = (np.arange(128)[:, None] * 32 + np.arange(OVC)[None, :])  # [128, 40]
    emb_ov = embp[:, idx, :].reshape(BPC, 128, OVC * D)

    # keys[bt, p, c] = g(code) for position j = 32p + c
    seq_c = seq_np[q0:q0 + BPC].astype(np.int64)  # [BPC, L]
    keys = np.empty((BPC * T, 128, NCH), np.float32)
    jpos = (np.arange(128)[:, None] * 32 + np.arange(NCH)[None, :])  # [128, 32]
    for q in range(BPC):
        for t in range(T):
            s = t + 1
            n_t = L - s
            a = seq_c[q]
            code = np.full(L, NBINS, np.int64)  # junk bin 400
            code[:n_t] = a[:n_t] * NUM_AA + a[s:s + n_t]
            keys[q * T + t] = _g(code)[jpos]
    keys = keys.astype(np.float32)

    iota_row = np.full(IOTA_W, 9999.0, np.float32)
    iota_row[:NCOLS] = _g(np.arange(NCOLS))
    iota = np.broadcast_to(iota_row, (128, IOTA_W)).astype(ml_dtypes.bfloat16)

    return {
        "emb_ov": np.ascontiguousarray(emb_ov),
        "keys": np.ascontiguousarray(keys),
        "iota": np.ascontiguousarray(iota),
    }


def kernel(seq, emb, k):
    seq_np = np.asarray(seq)
    emb_np = np.asarray(emb, dtype=np.float32)
    kk = int(np.asarray(k))
    assert kk + 1 == T and seq_np.shape == (B, L) and emb_np.shape == (B, L, D)

    from concourse.bass_utils import run_bass_kernel_spmd

    nc = _build()
    in_maps = [_host_prep(seq_np, emb_np, c) for c in range(NCORES)]
    import os
    trace = bool(int(os.environ.get("CK_TRACE", "0")))
    res = run_bass_kernel_spmd(nc, in_maps, list(range(NCORES)), trace=trace)
    global _LAST_EXEC_NS, _LAST_RES
    _LAST_EXEC_NS = res.exec_time_ns
    _LAST_RES = res

    out = np.empty((B, T, NUM_AA, NUM_AA, D), np.float32)
    for c in range(NCORES):
        o = res.results[c]["outp"]  # [nbt, D, NBINS]
        o = np.ascontiguousarray(o.transpose(0, 2, 1))  # [nbt, NBINS, D]
        out[c * BPC:(c + 1) * BPC] = o.reshape(BPC, T, NUM_AA, NUM_AA, D)
    return out


# revision 12
# speedup vs baseline: 1.0122x; 1.0122x over previous
"""CKSAAP embedding-average histogram kernel for Trainium2 (8 NeuronCores).

Problem: seq [B=32, L=4096] int codes in [0,20); emb [B, L, D=64] f32; k=7.
out[b, t, a, c, :] = (1/(L-t-1)) * sum_j 0.5*(emb[b,j]+emb[b,j+t+1])
                     over j with seq[b,j]==a and seq[b,j+t+1]==c.

Sharding: data-parallel over batch, 4 batches per core.

Device algorithm (per core), pure one-hot matmul formulation:
  For each (local batch q, gap t): V = emb[j] + emb[j+s]  (s=t+1, f32, DVE),
  split V into bf16 hi+lo halves (exact to ~2^-17), build the pair-code
  one-hot P [positions, 401] on DVE (bf16 is_equal against a remapped iota
  row; code 400 is a junk bin absorbing the invalid tail), then TensorE
  accumulates psum[128, 401] = [V_hi; V_lo]^T @ P over 32 position-chunks
  of 128 (chunk c = positions {32p + c}, so emb loads stay natural/contiguous
  with an 8-column overlap for the shifts).  Eviction sums hi+lo rows and
  scales by 0.5/(L-s) on ScalarE.  Output is written transposed [64, 400];
  the host restores [400, 64] ordering when unsharding.

Codes are remapped by g(n) = n (n<=255) else 256 + 4*(n-256) so every code
is exactly representable in bf16 (no is_equal collisions).
"""

import numpy as np
import ml_dtypes

NUM_AA = 20
L = 4096
D = 64
T = 8           # gaps t = 0..7 (k+1)
B = 32
NCORES = 8
BPC = B // NCORES   # batches per core
NBINS = NUM_AA * NUM_AA  # 400
NCH = 32        # contraction chunks of 128 positions
OVC = 40        # overlap columns per partition (32 + max shift 8)
NCOLS = NBINS  # invalid-position keys (g(400)=832) match no iota column
IOTA_W = 416    # iota row width; 32-elem-aligned rows keep the DVE packed mode

_CACHE = {}


def _g(n):
    """Injective map of codes 0..400 onto exactly-bf16-representable floats."""
    n = np.asarray(n, dtype=np.int64)
    return np.where(n <= 255, n, 256 + 4 * (n - 256)).astype(np.float32)


def _build(nbt=BPC * T):
    """Build the per-core Bass program. nbt = number of (batch, t) combos."""
    key = ("nc", nbt)
    if key in _CACHE:
        return _CACHE[key]
    import concourse.bass as bass
    import concourse.bacc as bacc
    import concourse.mybir as mybir
    from concourse import tile

    fp32 = mybir.dt.float32
    bf16 = mybir.dt.bfloat16
    AOP = mybir.AluOpType
    ACT = mybir.ActivationFunctionType

    nc = bacc.Bacc()
    emb_ov = nc.dram_tensor("emb_ov", [BPC, 128, OVC * D], fp32, kind="ExternalInput")
    keys = nc.dram_tensor("keys", [nbt, 128, NCH], fp32, kind="ExternalInput")
    iota = nc.dram_tensor("iota", [128, IOTA_W], bf16, kind="ExternalInput")
    outp = nc.dram_tensor("outp", [nbt, D, NBINS], fp32, kind="ExternalOutput")

    with tile.TileContext(nc) as tc:
        with (
            tc.tile_pool(name="const", bufs=1) as cpool,
            tc.tile_pool(name="work", bufs=2) as wpool,
            tc.tile_pool(name="vwork", bufs=3) as vpool,
            tc.tile_pool(name="outs", bufs=3) as opool,
            tc.tile_pool(name="psum", bufs=6, space="PSUM") as ppool,
        ):
            emb_sb = cpool.tile([128, BPC * OVC * D], fp32, tag="emb")
            keys_sb = cpool.tile([128, nbt, NCH], fp32, tag="keys")
            iota_sb = cpool.tile([128, IOTA_W], bf16, tag="iota")

            for q in range(BPC):
                nc.sync.dma_start(
                    emb_sb[:, q * OVC * D:(q + 1) * OVC * D], emb_ov[q]
                )
            nc.sync.dma_start(keys_sb[:], keys[:].rearrange("bt p c -> p bt c"))
            nc.sync.dma_start(iota_sb[:], iota[:])

            for bt in range(nbt):
                q, t = divmod(bt, T)
                s = t + 1
                n_t = L - s
                scale = float(0.5 / n_t)
                base = q * OVC * D

                vf = vpool.tile([128, NCH * D], fp32, tag="vf")
                vhl = vpool.tile([128, NCH, 128], bf16, tag="vhl")
                pall = wpool.tile([128, NCH, IOTA_W], bf16, tag="pall")

                # V = e_j + e_{j+s}; position j = 32p + c lives at [p, c*64:...]
                nc.vector.tensor_tensor(
                    vf[:],
                    emb_sb[:, base:base + NCH * D],
                    emb_sb[:, base + s * D:base + s * D + NCH * D],
                    AOP.add,
                )
                vf3 = vf[:].rearrange("p (c d) -> p c d", c=NCH)
                # hi = bf16(V) on ScalarE; lo = bf16(V - hi) on GpSimd
                nc.scalar.activation(vhl[:, :, 0:D], vf3, ACT.Copy)
                nc.gpsimd.tensor_tensor(
                    vhl[:, :, D:2 * D], vf3, vhl[:, :, 0:D], AOP.subtract
                )
                # P[p, c, n] = (iota[n] == keys[p, bt*NCH + c]) in bf16.
                # One tensor_scalar per chunk: single-src + bf16 + unit stride
                # hits the DVE packed mode (broadcast tensor_tensor runs 1x,
                # and a chunk-major layout would force a strided matmul rhs
                # that slows PE 3x).
                for c in range(NCH):
                    nc.vector.tensor_scalar(
                        pall[:, c, :],
                        iota_sb[:],
                        keys_sb[:, bt, c:c + 1],
                        None,
                        AOP.is_equal,
                    )

                ps = ppool.tile([128, NCOLS], fp32, tag="ps")
                for c in range(NCH):
                    nc.tensor.matmul(
                        ps[:],
                        vhl[:, c, :],
                        pall[:, c, 0:NCOLS],
                        start=(c == 0),
                        stop=(c == NCH - 1),
                    )

                # DVE cannot mix partition bases across operands; ScalarE can
                # read an offset partition base, so evict hi/lo separately
                # (scale folded into each) and add on DVE.
                his = opool.tile([D, NBINS], fp32, tag="his")
                los = opool.tile([D, NBINS], fp32, tag="los")
                oscaled = opool.tile([D, NBINS], fp32, tag="oscaled")
                nc.scalar.activation(his[:], ps[0:D, 0:NBINS], ACT.Copy, scale=scale)
                nc.scalar.activation(los[:], ps[D:128, 0:NBINS], ACT.Copy, scale=scale)
                nc.vector.tensor_tensor(oscaled[:], his[:], los[:], AOP.add)
                nc.sync.dma_start(outp[bt], oscaled[:])

    nc.finalize()
    _CACHE[key] = nc
    return nc


def _host_prep(seq_np, emb_np, core):
    """Build the per-core input arrays for core index `core`."""
    q0 = core * BPC
    emb_c = emb_np[q0:q0 + BPC]  # [BPC, L, D] f32
    # overlap layout: emb_ov[q, p, :] = emb[q, 32p : 32p+40, :] (zero-padded)
    embp = np.zeros((BPC, L + OVC - 32, D), np.float32)
    embp[:, :L] = emb_c
    idx = (np.arange(128)[:, None] * 32 + np.arange(OVC)[None, :])  # [128, 40]
    emb_ov = embp[:, idx, :].reshape(BPC, 128, OVC * D)

    # keys[bt, p, c] = g(code) for position j = 32p + c
    seq_c = seq_np[q0:q0 + BPC].astype(np.int64)  # [BPC, L]
    keys = np.empty((BPC * T, 128, NCH), np.float32)
    jpos = (np.arange(128)[:, None] * 32 + np.arange(NCH)[None, :])  # [128, 32]
    for q in range(BPC):
        for t in range(T):
            s = t + 1
            n_t = L - s
            a = seq_c[q]
            code = np.full(L, NBINS, np.int64)  # junk bin 400
            code[:n_t] = a[:n_t] * NUM_AA + a[s:s + n_t]
            keys[q * T + t] = _g(code)[jpos]
    keys = keys.astype(np.float32)

    iota_row = np.full(IOTA_W, 9999.0, np.float32)
    iota_row[:NBINS] = _g(np.arange(NBINS))
    iota = np.broadcast_to(iota_row, (128, IOTA_W)).astype(ml_dtypes.bfloat16)

    return {
        "emb_ov": np.ascontiguousarray(emb_ov),
        "keys": np.ascontiguousarray(keys),
        "iota": np.ascontiguousarray(iota),
    }


def kernel(seq, emb, k):
    seq_np = np.asarray(seq)
    emb_np = np.asarray(emb, dtype=np.float32)
    kk = int(np.asarray(k))
    assert kk + 1 == T and seq_np.shape == (B, L) and emb_np.shape == (B, L, D)

    from concourse.bass_utils import run_bass_kernel_spmd

    nc = _build()
    in_maps = [_host_prep(seq_np, emb_np, c) for c in range(NCORES)]
    import os
    trace = bool(int(os.environ.get("CK_TRACE", "0")))
    res = run_bass_kernel_spmd(nc, in_maps, list(range(NCORES)), trace=trace)
    global _LAST_EXEC_NS, _LAST_RES
    _LAST_EXEC_NS = res.exec_time_ns
    _LAST_RES = res

    out = np.empty((B, T, NUM_AA, NUM_AA, D), np.float32)
    for c in range(NCORES):
        o = res.results[c]["outp"]  # [nbt, D, NBINS]
        o = np.ascontiguousarray(o.transpose(0, 2, 1))  # [nbt, NBINS, D]
        out[c * BPC:(c + 1) * BPC] = o.reshape(BPC, T, NUM_AA, NUM_AA, D)
    return out


# revision 14
# speedup vs baseline: 1.2600x; 1.2447x over previous
"""CKSAAP embedding-average histogram kernel for Trainium2 (8 NeuronCores).

Problem: seq [B=32, L=4096] int codes in [0,20); emb [B, L, D=64] f32; k=7.
out[b, t, a, c, :] = (1/(L-t-1)) * sum_j 0.5*(emb[b,j]+emb[b,j+t+1])
                     over j with seq[b,j]==a and seq[b,j+t+1]==c.

Sharding: data-parallel over batch, 4 batches per core.

Device algorithm (per core), pure one-hot matmul formulation:
  For each (local batch q, gap t): V = emb[j] + emb[j+s]  (s=t+1, f32, DVE),
  split V into bf16 hi+lo halves (exact to ~2^-17), build the pair-code
  one-hot P [positions, 401] on DVE (bf16 is_equal against a remapped iota
  row; code 400 is a junk bin absorbing the invalid tail), then TensorE
  accumulates psum[128, 401] = [V_hi; V_lo]^T @ P over 32 position-chunks
  of 128 (chunk c = positions {32p + c}, so emb loads stay natural/contiguous
  with an 8-column overlap for the shifts).  Eviction sums hi+lo rows and
  scales by 0.5/(L-s) on ScalarE.  Output is written transposed [64, 400];
  the host restores [400, 64] ordering when unsharding.

Codes are remapped by g(n) = n (n<=255) else 256 + 4*(n-256) so every code
is exactly representable in bf16 (no is_equal collisions).
"""

import numpy as np
import ml_dtypes

NUM_AA = 20
L = 4096
D = 64
T = 8           # gaps t = 0..7 (k+1)
B = 32
NCORES = 8
BPC = B // NCORES   # batches per core
NBINS = NUM_AA * NUM_AA  # 400
NCH = 32        # contraction chunks of 128 positions
NCHX = 40       # key slots: 32 chunks + 8 boundary (pre-shifted) slots
OVC = 40        # overlap columns per partition (32 + max shift 8)
NCOLS = NBINS  # invalid-position keys (g(400)=832) match no iota column
IOTA_W = 416    # iota row width; 32-elem-aligned rows keep the DVE packed mode

_CACHE = {}


def _g(n):
    """Injective map of codes 0..400 onto exactly-bf16-representable floats."""
    n = np.asarray(n, dtype=np.int64)
    return np.where(n <= 255, n, 256 + 4 * (n - 256)).astype(np.float32)


def _build(nbt=BPC * T):
    """Build the per-core Bass program. nbt = number of (batch, t) combos."""
    key = ("nc", nbt)
    if key in _CACHE:
        return _CACHE[key]
    import concourse.bass as bass
    import concourse.bacc as bacc
    import concourse.mybir as mybir
    from concourse import tile

    fp32 = mybir.dt.float32
    bf16 = mybir.dt.bfloat16
    AOP = mybir.AluOpType
    ACT = mybir.ActivationFunctionType

    nc = bacc.Bacc()
    emb_hl = nc.dram_tensor("emb_hl", [BPC, 128, NCH * 2 * D], bf16, kind="ExternalInput")
    keys = nc.dram_tensor("keys", [nbt, 128, NCHX], fp32, kind="ExternalInput")
    iota = nc.dram_tensor("iota", [128, IOTA_W], bf16, kind="ExternalInput")
    outp = nc.dram_tensor("outp", [nbt, D, NBINS], fp32, kind="ExternalOutput")

    with tile.TileContext(nc) as tc:
        with (
            tc.tile_pool(name="const", bufs=1) as cpool,
            tc.tile_pool(name="work", bufs=2) as wpool,
            tc.tile_pool(name="vwork", bufs=3) as vpool,
            tc.tile_pool(name="outs", bufs=3) as opool,
            tc.tile_pool(name="psum", bufs=6, space="PSUM") as ppool,
        ):
            emb_sb = cpool.tile([128, BPC, NCH, 2 * D], bf16, tag="emb")
            keys_sb = cpool.tile([128, nbt, NCHX], fp32, tag="keys")
            iota_sb = cpool.tile([128, IOTA_W], bf16, tag="iota")

            for q in range(BPC):
                nc.sync.dma_start(
                    emb_sb[:, q], emb_hl[q].rearrange("p (c x) -> p c x", c=NCH)
                )
            nc.sync.dma_start(keys_sb[:], keys[:].rearrange("bt p c -> p bt c"))
            nc.sync.dma_start(iota_sb[:], iota[:])

            for bt in range(nbt):
                q, t = divmod(bt, T)
                s = t + 1
                n_t = L - s
                scale = float(0.5 / n_t)
                base = q * OVC * D

                pall = wpool.tile([128, NCHX, IOTA_W], bf16, tag="pall")
                # P[p, c, n] = (iota[n] == keys[p, bt*NCH + c]) in bf16.
                # One tensor_scalar per chunk: single-src + bf16 + unit stride
                # hits the DVE packed mode (broadcast tensor_tensor runs 1x,
                # and a chunk-major layout would force a strided matmul rhs
                # that slows PE 3x).
                for c in list(range(NCH)) + list(range(NCH, NCH + s)):
                    nc.vector.tensor_scalar(
                        pall[:, c, :],
                        iota_sb[:],
                        keys_sb[:, bt, c:c + 1],
                        None,
                        AOP.is_equal,
                    )

                # out = sum_j e_j (x) (P[j] + P[j-s]): stream each chunk's
                # one-hot twice against the hi|lo weights of e_j (no V tensor
                # is ever built).  For c < s the shifted term pairs partition
                # p with p-1, handled by a K=127 matmul at partition offset 1.
                ps = ppool.tile([128, NCOLS], fp32, tag="ps")
                for c in range(NCH):
                    w = emb_sb[:, q, c, :]
                    nc.tensor.matmul(
                        ps[:], w, pall[:, c, 0:NCOLS],
                        start=(c == 0), stop=False,
                    )
                    c2 = c - s if c >= s else NCH + c
                    nc.tensor.matmul(
                        ps[:], w, pall[:, c2, 0:NCOLS],
                        start=False, stop=(c == NCH - 1),
                    )

                # DVE cannot mix partition bases across operands; ScalarE can
                # read an offset partition base, so evict hi/lo separately
                # (scale folded into each) and add on DVE.
                his = opool.tile([D, NBINS], fp32, tag="his")
                los = opool.tile([D, NBINS], fp32, tag="los")
                oscaled = opool.tile([D, NBINS], fp32, tag="oscaled")
                nc.scalar.activation(his[:], ps[0:D, 0:NBINS], ACT.Copy, scale=scale)
                nc.scalar.activation(los[:], ps[D:128, 0:NBINS], ACT.Copy, scale=scale)
                nc.vector.tensor_tensor(oscaled[:], his[:], los[:], AOP.add)
                nc.sync.dma_start(outp[bt], oscaled[:])

    nc.finalize()
    _CACHE[key] = nc
    return nc


def _host_prep(seq_np, emb_np, core):
    """Build the per-core input arrays for core index `core`."""
    q0 = core * BPC
    emb_c = emb_np[q0:q0 + BPC]  # [BPC, L, D] f32
    # lossless bf16 hi|lo split of e, natural layout [p, c, hi(64)|lo(64)]
    e_hi = emb_c.astype(ml_dtypes.bfloat16)
    e_lo = (emb_c - e_hi.astype(np.float32)).astype(ml_dtypes.bfloat16)
    ehl = np.concatenate([e_hi, e_lo], axis=-1)  # [BPC, L, 128]
    emb_hl = ehl.reshape(BPC, 128, NCH, 2 * D).reshape(BPC, 128, NCH * 2 * D)

    # keys[bt, p, c] = g(code) for position j = 32p + c
    seq_c = seq_np[q0:q0 + BPC].astype(np.int64)  # [BPC, L]
    keys = np.full((BPC * T, 128, NCHX), 832.0, np.float32)
    jpos = (np.arange(128)[:, None] * 32 + np.arange(NCH)[None, :])  # [128, 32]
    for q in range(BPC):
        for t in range(T):
            s = t + 1
            n_t = L - s
            a = seq_c[q]
            code = np.full(L, NBINS, np.int64)  # invalid -> no iota match
            code[:n_t] = a[:n_t] * NUM_AA + a[s:s + n_t]
            g = _g(code)
            keys[q * T + t, :, :NCH] = g[jpos]
            # boundary slots: key(32p + cb - s) for cb < s (p=0 row invalid)
            for cb in range(s):
                j2 = np.arange(128) * 32 + cb - s
                valid = j2 >= 0
                keys[q * T + t, valid, NCH + cb] = g[j2[valid]]
    keys = keys.astype(np.float32)

    iota_row = np.full(IOTA_W, 9999.0, np.float32)
    iota_row[:NBINS] = _g(np.arange(NBINS))
    iota = np.broadcast_to(iota_row, (128, IOTA_W)).astype(ml_dtypes.bfloat16)

    return {
        "emb_hl": np.ascontiguousarray(emb_hl),
        "keys": np.ascontiguousarray(keys),
        "iota": np.ascontiguousarray(iota),
    }


def kernel(seq, emb, k):
    seq_np = np.asarray(seq)
    emb_np = np.asarray(emb, dtype=np.float32)
    kk = int(np.asarray(k))
    assert kk + 1 == T and seq_np.shape == (B, L) and emb_np.shape == (B, L, D)

    from concourse.bass_utils import run_bass_kernel_spmd

    nc = _build()
    in_maps = [_host_prep(seq_np, emb_np, c) for c in range(NCORES)]
    import os
    trace = bool(int(os.environ.get("CK_TRACE", "0")))
    res = run_bass_kernel_spmd(nc, in_maps, list(range(NCORES)), trace=trace)
    global _LAST_EXEC_NS, _LAST_RES
    _LAST_EXEC_NS = res.exec_time_ns
    _LAST_RES = res

    out = np.empty((B, T, NUM_AA, NUM_AA, D), np.float32)
    for c in range(NCORES):
        o = res.results[c]["outp"]  # [nbt, D, NBINS]
        o = np.ascontiguousarray(o.transpose(0, 2, 1))  # [nbt, NBINS, D]
        out[c * BPC:(c + 1) * BPC] = o.reshape(BPC, T, NUM_AA, NUM_AA, D)
    return out


# revision 17
# speedup vs baseline: 1.3265x; 1.0528x over previous
"""CKSAAP embedding-average histogram kernel for Trainium2 (8 NeuronCores).

Problem: seq [B=32, L=4096] int codes in [0,20); emb [B, L, D=64] f32; k=7.
out[b, t, a, c, :] = (1/(L-t-1)) * sum_j 0.5*(emb[b,j]+emb[b,j+t+1])
                     over j with seq[b,j]==a and seq[b,j+t+1]==c.

Sharding: data-parallel over batch, 4 batches per core.

Device algorithm (per core), one-hot matmul with a double-streamed rhs:
  out_t = sum_j (e_j + e_{j+s}) (x) P_t[j]  (s = t+1) is computed WITHOUT
  ever materializing the pair values: by distributivity it equals
  sum_j e_j (x) (P_t[j] + P_t[j-s]), so TensorE streams each chunk's
  one-hot twice (slices c and c-s) against stationary weights e_j,
  accumulating in PSUM.  emb is uploaded as a lossless bf16 hi|lo pair
  (e = e_hi + e_lo exactly, same bytes as f32), giving M=128 weights
  [e_hi | e_lo] and ~1e-5 overall precision with zero vector-engine work.

  The pair-code one-hot P [positions, 400] is built on DVE with one
  tensor_scalar is_equal per 128-position chunk (single-source + bf16 +
  unit stride hits the packed perf mode; broadcast tensor_tensor runs 1x,
  and a chunk-major layout would force a strided matmul rhs that slows
  PE 3x).  Chunk c = positions {32p + c}: emb loads stay natural and
  contiguous.  For c < s the shifted term would pair partition p with
  p-1 (illegal for PE), so 8 extra pre-shifted boundary key slots per
  (batch, t) are uploaded and compared like normal chunks.  Invalid tail
  positions get key g(400)=832 which matches no iota column.  Eviction
  folds the hi and lo PSUM halves with the 0.5/(L-s) scale on ScalarE
  plus one DVE add; output is written transposed [64, 400] and the host
  restores [400, 64] ordering when unsharding.

Codes are remapped by g(n) = n (n<=255) else 256 + 4*(n-256) so every code
is exactly representable in bf16 (no is_equal collisions).
"""

import numpy as np
import ml_dtypes

NUM_AA = 20
L = 4096
D = 64
T = 8           # gaps t = 0..7 (k+1)
B = 32
NCORES = 8
BPC = B // NCORES   # batches per core
NBINS = NUM_AA * NUM_AA  # 400
NCH = 32        # contraction chunks of 128 positions
NCHX = 32       # key slots per (batch, t): one per chunk
OVC = 40        # overlap columns per partition (32 + max shift 8)
NCOLS = NBINS  # invalid-position keys (g(400)=832) match no iota column
IOTA_W = 416    # iota row width; 32-elem-aligned rows keep the DVE packed mode

_CACHE = {}


def _g(n):
    """Injective map of codes 0..400 onto exactly-bf16-representable floats."""
    n = np.asarray(n, dtype=np.int64)
    return np.where(n <= 255, n, 256 + 4 * (n - 256)).astype(np.float32)


def _build(nbt=BPC * T):
    """Build the per-core Bass program. nbt = number of (batch, t) combos."""
    key = ("nc", nbt)
    if key in _CACHE:
        return _CACHE[key]
    import concourse.bass as bass
    import concourse.bacc as bacc
    import concourse.mybir as mybir
    from concourse import tile

    fp32 = mybir.dt.float32
    bf16 = mybir.dt.bfloat16
    AOP = mybir.AluOpType
    ACT = mybir.ActivationFunctionType

    nc = bacc.Bacc()
    emb_hl = nc.dram_tensor("emb_hl", [BPC, 128, OVC * 2 * D], bf16, kind="ExternalInput")
    emb_ov32 = nc.dram_tensor("emb_ov32", [128, OVC * D], fp32, kind="ExternalInput")
    keys = nc.dram_tensor("keys", [nbt, 128, NCHX], fp32, kind="ExternalInput")
    iota = nc.dram_tensor("iota", [128, IOTA_W], bf16, kind="ExternalInput")
    outp = nc.dram_tensor("outp", [nbt, D, NBINS], fp32, kind="ExternalOutput")

    with tile.TileContext(nc) as tc:
        with (
            tc.tile_pool(name="const", bufs=1) as cpool,
            tc.tile_pool(name="work", bufs=2) as wpool,
            tc.tile_pool(name="vwork", bufs=3) as vpool,
            tc.tile_pool(name="outs", bufs=3) as opool,
            tc.tile_pool(name="psum", bufs=6, space="PSUM") as ppool,
        ):
            emb_sb = cpool.tile([128, BPC, OVC, 2 * D], bf16, tag="emb")
            emb32_sb = cpool.tile([128, OVC * D], fp32, tag="emb32")
            keys_sb = cpool.tile([128, nbt, NCHX], fp32, tag="keys")
            iota_sb = cpool.tile([128, IOTA_W], bf16, tag="iota")

            for q in range(BPC):
                nc.sync.dma_start(
                    emb_sb[:, q], emb_hl[q].rearrange("p (c x) -> p c x", c=OVC)
                )
            nc.sync.dma_start(emb32_sb[:], emb_ov32[:])
            nc.sync.dma_start(keys_sb[:], keys[:].rearrange("bt p c -> p bt c"))
            nc.sync.dma_start(iota_sb[:], iota[:])

            bt_order = [q * T + t for t in range(T) for q in range(BPC)]
            if nbt != BPC * T:
                bt_order = list(range(nbt))
            for bt in bt_order:
                q, t = divmod(bt, T)
                s = t + 1
                n_t = L - s
                scale = float(0.5 / n_t)
                base = q * OVC * D

                pall = wpool.tile([128, NCHX, IOTA_W], bf16, tag="pall")
                # P[p, c, n] = (iota[n] == keys[p, bt*NCH + c]) in bf16.
                # One tensor_scalar per chunk: single-src + bf16 + unit stride
                # hits the DVE packed mode (broadcast tensor_tensor runs 1x,
                # and a chunk-major layout would force a strided matmul rhs
                # that slows PE 3x).
                for c in range(NCH):
                    nc.vector.tensor_scalar(
                        pall[:, c, :],
                        iota_sb[:],
                        keys_sb[:, bt, c:c + 1],
                        None,
                        AOP.is_equal,
                    )

                ps = ppool.tile([128, NCOLS], fp32, tag="ps")
                if q == 0:
                    # Hybrid V-path for one batch: build V = e_j + e_{j+s}
                    # explicitly (f32 add on DVE, hi-cast on ScalarE, lo on
                    # GpSimd) and stream each one-hot ONCE.  This trades spare
                    # DVE/ScalarE/GpSimd capacity for PE stream time.
                    vf = vpool.tile([128, NCH * D], fp32, tag="vf")
                    vhl = vpool.tile([128, NCH, 128], bf16, tag="vhl")
                    nc.vector.tensor_tensor(
                        vf[:],
                        emb32_sb[:, 0:NCH * D],
                        emb32_sb[:, s * D:s * D + NCH * D],
                        AOP.add,
                    )
                    vf3 = vf[:].rearrange("p (c d) -> p c d", c=NCH)
                    nc.scalar.activation(vhl[:, :, 0:D], vf3, ACT.Copy)
                    nc.gpsimd.tensor_tensor(
                        vhl[:, :, D:2 * D], vf3, vhl[:, :, 0:D], AOP.subtract
                    )
                    for c in range(NCH):
                        nc.tensor.matmul(
                            ps[:], vhl[:, c, :], pall[:, c, 0:NCOLS],
                            start=(c == 0), stop=(c == NCH - 1),
                        )
                else:
                    # out = sum_j e_j (x) (P[j] + P[j-s]) = term1 + term2 with
                    # term2 re-chunked by m' = m - s, so its weights are the
                    # SHIFTED emb slice (within-partition via the 8-column
                    # overlap) and every rhs is a plain chunk slice.
                    for c in range(NCH):
                        nc.tensor.matmul(
                            ps[:], emb_sb[:, q, c, :], pall[:, c, 0:NCOLS],
                            start=(c == 0), stop=False,
                        )
                        nc.tensor.matmul(
                            ps[:], emb_sb[:, q, c + s, :], pall[:, c, 0:NCOLS],
                            start=False, stop=(c == NCH - 1),
                        )

                # DVE cannot mix partition bases across operands; ScalarE can
                # read an offset partition base, so evict hi/lo separately
                # (scale folded into each) and add on DVE.
                his = opool.tile([D, NBINS], fp32, tag="his")
                los = opool.tile([D, NBINS], fp32, tag="los")
                oscaled = opool.tile([D, NBINS], fp32, tag="oscaled")
                nc.scalar.activation(his[:], ps[0:D, 0:NBINS], ACT.Copy, scale=scale)
                nc.scalar.activation(los[:], ps[D:128, 0:NBINS], ACT.Copy, scale=scale)
                nc.vector.tensor_tensor(oscaled[:], his[:], los[:], AOP.add)
                nc.sync.dma_start(outp[bt], oscaled[:])

    nc.finalize()
    _CACHE[key] = nc
    return nc


def _host_prep(seq_np, emb_np, core):
    """Build the per-core input arrays for core index `core`."""
    q0 = core * BPC
    emb_c = emb_np[q0:q0 + BPC]  # [BPC, L, D] f32
    # lossless bf16 hi|lo split of e, overlap layout: partition p holds
    # positions 32p .. 32p+39 (8 extra for the shifted term2 weights)
    embp = np.zeros((BPC, L + OVC - 32, D), np.float32)
    embp[:, :L] = emb_c
    e_hi = embp.astype(ml_dtypes.bfloat16)
    e_lo = (embp - e_hi.astype(np.float32)).astype(ml_dtypes.bfloat16)
    ehl = np.concatenate([e_hi, e_lo], axis=-1)  # [BPC, L+8, 128]
    idx = (np.arange(128)[:, None] * 32 + np.arange(OVC)[None, :])  # [128, 40]
    emb_hl = ehl[:, idx, :].reshape(BPC, 128, OVC * 2 * D)
    emb_ov32 = embp[0][idx, :].reshape(128, OVC * D)  # f32, batch q=0 only

    # keys[bt, p, c] = g(code) for position j = 32p + c
    seq_c = seq_np[q0:q0 + BPC].astype(np.int64)  # [BPC, L]
    keys = np.full((BPC * T, 128, NCHX), 832.0, np.float32)
    jpos = (np.arange(128)[:, None] * 32 + np.arange(NCH)[None, :])  # [128, 32]
    for q in range(BPC):
        for t in range(T):
            s = t + 1
            n_t = L - s
            a = seq_c[q]
            code = np.full(L, NBINS, np.int64)  # invalid -> no iota match
            code[:n_t] = a[:n_t] * NUM_AA + a[s:s + n_t]
            keys[q * T + t] = _g(code)[jpos]

    iota_row = np.full(IOTA_W, 9999.0, np.float32)
    iota_row[:NBINS] = _g(np.arange(NBINS))
    iota = np.broadcast_to(iota_row, (128, IOTA_W)).astype(ml_dtypes.bfloat16)

    return {
        "emb_hl": np.ascontiguousarray(emb_hl),
        "emb_ov32": np.ascontiguousarray(emb_ov32),
        "keys": np.ascontiguousarray(keys),
        "iota": np.ascontiguousarray(iota),
    }


def kernel(seq, emb, k):
    seq_np = np.asarray(seq)
    emb_np = np.asarray(emb, dtype=np.float32)
    kk = int(np.asarray(k))
    assert kk + 1 == T and seq_np.shape == (B, L) and emb_np.shape == (B, L, D)

    from concourse.bass_utils import run_bass_kernel_spmd

    nc = _build()
    in_maps = [_host_prep(seq_np, emb_np, c) for c in range(NCORES)]
    import os
    trace = bool(int(os.environ.get("CK_TRACE", "0")))
    res = run_bass_kernel_spmd(nc, in_maps, list(range(NCORES)), trace=trace)
    global _LAST_EXEC_NS, _LAST_RES
    _LAST_EXEC_NS = res.exec_time_ns
    _LAST_RES = res

    out = np.empty((B, T, NUM_AA, NUM_AA, D), np.float32)
    for c in range(NCORES):
        o = res.results[c]["outp"]  # [nbt, D, NBINS]
        o = np.ascontiguousarray(o.transpose(0, 2, 1))  # [nbt, NBINS, D]
        out[c * BPC:(c + 1) * BPC] = o.reshape(BPC, T, NUM_AA, NUM_AA, D)
    return out


# revision 23
# speedup vs baseline: 1.3703x; 1.0330x over previous
"""CKSAAP embedding-average histogram kernel for Trainium2 (8 NeuronCores).

Problem: seq [B=32, L=4096] int codes in [0,20); emb [B, L, D=64] f32; k=7.
out[b, t, a, c, :] = (1/(L-t-1)) * sum_j 0.5*(emb[b,j]+emb[b,j+t+1])
                     over j with seq[b,j]==a and seq[b,j+t+1]==c.

Sharding: data-parallel over batch, 4 batches per core.

Device algorithm (per core), one-hot matmul, hybrid of two streaming modes:
  out_t = sum_j (e_j + e_{j+s}) (x) P_t[j]  (s = t+1, P_t = pair-code
  one-hot).  The pair-code one-hot P [positions, 400] is built on DVE with
  one tensor_scalar is_equal per 128-position chunk (single-source + bf16
  + unit stride hits the packed perf mode; broadcast tensor_tensor runs
  1x, and a chunk-major layout would force a strided matmul rhs that
  slows PE 3x).  Chunk c = positions {32p + c}: emb loads stay natural
  and contiguous, with an 8-column overlap so every shifted read is
  within-partition.  Invalid tail positions get key g(400)=832 which
  matches no iota column.

  Double-stream mode (batches q=1..3): by distributivity the output is
  sum_j e_j (x) P[j] + sum_{m'} e_{m'+s} (x) P[m'], so TensorE streams
  each chunk's one-hot twice -- once against weights e_j, once against
  the overlap-shifted weights e_{j+s} -- accumulating in PSUM.  The pair
  values are never materialized and emb is uploaded as a lossless bf16
  hi|lo pair (e = e_hi + e_lo exactly, same bytes as f32), giving M=128
  weights [e_hi | e_lo] with no vector-engine cost.

  V-path mode (batch q=0, interleaved t-major so the pipeline mixes both
  kinds): V = e_j + e_{j+s} is built explicitly (f32 add on DVE, bf16 hi
  cast on ScalarE, lo = V - hi on GpSimd) and each one-hot streams ONCE.
  This trades spare DVE/ScalarE/GpSimd capacity for PE stream time; one
  batch is the measured balance point (DVE 86% vs PE 84% busy).

  Eviction folds the hi and lo PSUM halves with the 0.5/(L-s) scale on
  ScalarE plus one DVE add; output is written transposed [64, 400] and
  the host restores [400, 64] ordering when unsharding.

Codes are remapped by g(n) = n (n<=255) else 256 + 4*(n-256) so every code
is exactly representable in bf16 (no is_equal collisions).
"""

import numpy as np
import ml_dtypes

NUM_AA = 20
L = 4096
D = 64
T = 8           # gaps t = 0..7 (k+1)
B = 32
NCORES = 8
BPC = B // NCORES   # batches per core
NBINS = NUM_AA * NUM_AA  # 400
NCH = 32        # contraction chunks of 128 positions
NCHX = 32       # key slots per (batch, t): one per chunk
OVC = 40        # overlap columns per partition (32 + max shift 8)
NCOLS = NBINS  # invalid-position keys (g(400)=832) match no iota column
IOTA_W = 416    # iota row width; 32-elem-aligned rows keep the DVE packed mode

_CACHE = {}


def _g(n):
    """Injective map of codes 0..400 onto exactly-bf16-representable floats."""
    n = np.asarray(n, dtype=np.int64)
    return np.where(n <= 255, n, 256 + 4 * (n - 256)).astype(np.float32)


def _build(nbt=BPC * T):
    """Build the per-core Bass program. nbt = number of (batch, t) combos."""
    key = ("nc", nbt)
    if key in _CACHE:
        return _CACHE[key]
    import concourse.bass as bass
    import concourse.bacc as bacc
    import concourse.mybir as mybir
    from concourse import tile

    fp32 = mybir.dt.float32
    bf16 = mybir.dt.bfloat16
    AOP = mybir.AluOpType
    ACT = mybir.ActivationFunctionType

    nc = bacc.Bacc()
    emb_hl = nc.dram_tensor("emb_hl", [BPC, 128, OVC * 2 * D], bf16, kind="ExternalInput")
    emb_ov32 = nc.dram_tensor("emb_ov32", [128, OVC * D], fp32, kind="ExternalInput")
    keys = nc.dram_tensor("keys", [nbt, 128, NCHX], fp32, kind="ExternalInput")
    iota = nc.dram_tensor("iota", [128, IOTA_W], bf16, kind="ExternalInput")
    outp = nc.dram_tensor("outp", [nbt, D, NBINS], fp32, kind="ExternalOutput")

    with tile.TileContext(nc) as tc:
        with (
            tc.tile_pool(name="const", bufs=1) as cpool,
            tc.tile_pool(name="work", bufs=3) as wpool,
            tc.tile_pool(name="vwork", bufs=2) as vpool,
            tc.tile_pool(name="outs", bufs=2) as opool,
            tc.tile_pool(name="psum", bufs=8, space="PSUM") as ppool,
        ):
            emb_sb = cpool.tile([128, BPC, OVC, 2 * D], bf16, tag="emb")
            emb32_sb = cpool.tile([128, OVC * D], fp32, tag="emb32")
            keys_sb = cpool.tile([128, nbt, NCHX], fp32, tag="keys")
            iota_sb = cpool.tile([128, IOTA_W], bf16, tag="iota")

            for q in range(BPC):
                nc.sync.dma_start(
                    emb_sb[:, q], emb_hl[q].rearrange("p (c x) -> p c x", c=OVC)
                )
            nc.sync.dma_start(emb32_sb[:], emb_ov32[:])
            nc.sync.dma_start(keys_sb[:], keys[:].rearrange("bt p c -> p bt c"))
            nc.sync.dma_start(iota_sb[:], iota[:])

            bt_order = [q * T + t for t in range(T) for q in range(BPC)]
            if nbt != BPC * T:
                bt_order = list(range(nbt))
            for bt in bt_order:
                q, t = divmod(bt, T)
                s = t + 1
                n_t = L - s
                scale = float(0.5 / n_t)
                base = q * OVC * D

                # P[p, c, n] = (iota[n] == keys[p, bt*NCH + c]) in bf16.
                # One tensor_scalar per chunk: single-src + bf16 + unit stride
                # hits the DVE packed mode (broadcast tensor_tensor runs 1x,
                # and a chunk-major layout would force a strided matmul rhs
                # that slows PE 3x).  Two half-tiles so PE can stream half A
                # while DVE still generates half B.
                H = NCH // 2
                pall_a = wpool.tile([128, H, IOTA_W], bf16, tag="palla")
                pall_b = wpool.tile([128, H, IOTA_W], bf16, tag="pallb")
                def pslice(c):
                    return pall_a[:, c, 0:NCOLS] if c < H else pall_b[:, c - H, 0:NCOLS]
                for c in range(NCH):
                    nc.vector.tensor_scalar(
                        (pall_a if c < H else pall_b)[:, c % H, :],
                        iota_sb[:],
                        keys_sb[:, bt, c:c + 1],
                        None,
                        AOP.is_equal,
                    )

                ps = ppool.tile([128, NCOLS], fp32, tag="ps")
                if q == 0:
                    # Hybrid V-path for one batch: build V = e_j + e_{j+s}
                    # explicitly (f32 add on DVE, hi-cast on ScalarE, lo on
                    # GpSimd) and stream each one-hot ONCE.  This trades spare
                    # DVE/ScalarE/GpSimd capacity for PE stream time.
                    vf = vpool.tile([128, NCH * D], fp32, tag="vf")
                    vhl = vpool.tile([128, NCH, 128], bf16, tag="vhl")
                    nc.vector.tensor_tensor(
                        vf[:],
                        emb32_sb[:, 0:NCH * D],
                        emb32_sb[:, s * D:s * D + NCH * D],
                        AOP.add,
                    )
                    vf3 = vf[:].rearrange("p (c d) -> p c d", c=NCH)
                    nc.scalar.activation(vhl[:, :, 0:D], vf3, ACT.Copy)
                    nc.gpsimd.tensor_tensor(
                        vhl[:, :, D:2 * D], vf3, vhl[:, :, 0:D], AOP.subtract
                    )
                    for c in range(NCH):
                        nc.tensor.matmul(
                            ps[:], vhl[:, c, :], pslice(c),
                            start=(c == 0), stop=(c == NCH - 1),
                        )
                else:
                    # out = sum_j e_j (x) (P[j] + P[j-s]) = term1 + term2 with
                    # term2 re-chunked by m' = m - s, so its weights are the
                    # SHIFTED emb slice (within-partition via the 8-column
                    # overlap) and every rhs is a plain chunk slice.
                    for c in range(NCH):
                        nc.tensor.matmul(
                            ps[:], emb_sb[:, q, c, :], pslice(c),
                            start=(c == 0), stop=False,
                        )
                        nc.tensor.matmul(
                            ps[:], emb_sb[:, q, c + s, :], pslice(c),
                            start=False, stop=(c == NCH - 1),
                        )

                # DVE cannot mix partition bases across operands; ScalarE can
                # read an offset partition base, so evict hi/lo separately
                # (scale folded into each) and add on DVE.
                his = opool.tile([D, NBINS], fp32, tag="his")
                los = opool.tile([D, NBINS], fp32, tag="los")
                oscaled = opool.tile([D, NBINS], fp32, tag="oscaled")
                nc.scalar.activation(his[:], ps[0:D, 0:NBINS], ACT.Copy, scale=scale)
                nc.scalar.activation(los[:], ps[D:128, 0:NBINS], ACT.Copy, scale=scale)
                nc.vector.tensor_tensor(oscaled[:], his[:], los[:], AOP.add)
                nc.sync.dma_start(outp[bt], oscaled[:])

    nc.finalize()
    _CACHE[key] = nc
    return nc


def _host_prep(seq_np, emb_np, core):
    """Build the per-core input arrays for core index `core`."""
    q0 = core * BPC
    emb_c = emb_np[q0:q0 + BPC]  # [BPC, L, D] f32
    # lossless bf16 hi|lo split of e, overlap layout: partition p holds
    # positions 32p .. 32p+39 (8 extra for the shifted term2 weights)
    embp = np.zeros((BPC, L + OVC - 32, D), np.float32)
    embp[:, :L] = emb_c
    e_hi = embp.astype(ml_dtypes.bfloat16)
    e_lo = (embp - e_hi.astype(np.float32)).astype(ml_dtypes.bfloat16)
    ehl = np.concatenate([e_hi, e_lo], axis=-1)  # [BPC, L+8, 128]
    idx = (np.arange(128)[:, None] * 32 + np.arange(OVC)[None, :])  # [128, 40]
    emb_hl = ehl[:, idx, :].reshape(BPC, 128, OVC * 2 * D)
    emb_ov32 = embp[0][idx, :].reshape(128, OVC * D)  # f32, batch q=0 only

    # keys[bt, p, c] = g(code) for position j = 32p + c
    seq_c = seq_np[q0:q0 + BPC].astype(np.int64)  # [BPC, L]
    keys = np.full((BPC * T, 128, NCHX), 832.0, np.float32)
    jpos = (np.arange(128)[:, None] * 32 + np.arange(NCH)[None, :])  # [128, 32]
    for q in range(BPC):
        for t in range(T):
            s = t + 1
            n_t = L - s
            a = seq_c[q]
            code = np.full(L, NBINS, np.int64)  # invalid -> no iota match
            code[:n_t] = a[:n_t] * NUM_AA + a[s:s + n_t]
            keys[q * T + t] = _g(code)[jpos]

    iota_row = np.full(IOTA_W, 9999.0, np.float32)
    iota_row[:NBINS] = _g(np.arange(NBINS))
    iota = np.broadcast_to(iota_row, (128, IOTA_W)).astype(ml_dtypes.bfloat16)

    return {
        "emb_hl": np.ascontiguousarray(emb_hl),
        "emb_ov32": np.ascontiguousarray(emb_ov32),
        "keys": np.ascontiguousarray(keys),
        "iota": np.ascontiguousarray(iota),
    }


def kernel(seq, emb, k):
    seq_np = np.asarray(seq)
    emb_np = np.asarray(emb, dtype=np.float32)
    kk = int(np.asarray(k))
    assert kk + 1 == T and seq_np.shape == (B, L) and emb_np.shape == (B, L, D)

    from concourse.bass_utils import run_bass_kernel_spmd

    nc = _build()
    in_maps = [_host_prep(seq_np, emb_np, c) for c in range(NCORES)]
    import os
    trace = bool(int(os.environ.get("CK_TRACE", "0")))
    res = run_bass_kernel_spmd(nc, in_maps, list(range(NCORES)), trace=trace)
    global _LAST_EXEC_NS, _LAST_RES
    _LAST_EXEC_NS = res.exec_time_ns
    _LAST_RES = res

    out = np.empty((B, T, NUM_AA, NUM_AA, D), np.float32)
    for c in range(NCORES):
        o = res.results[c]["outp"]  # [nbt, D, NBINS]
        o = np.ascontiguousarray(o.transpose(0, 2, 1))  # [nbt, NBINS, D]
        out[c * BPC:(c + 1) * BPC] = o.reshape(BPC, T, NUM_AA, NUM_AA, D)
    return out
